# revision 15
# baseline (speedup 1.0000x reference)
"""DeepSeek-V2 normal MoE layer on 8 Trainium2 NeuronCores.

Expert-parallel sharding: core e holds expert e's weights (cast to bf16).
The router (tiny [T,E] matmul + softmax + top-k) runs on the host in fp32 —
this is the dispatch step of the sharding layer: it decides which token rows
are copied to which core. Each core receives its routed tokens twice
(plain, and pre-scaled by the renormalized top-k combine weight, which folds
the output weighting into the linear up-projection). On device, each core
computes gated-SiLU expert MLP for its tokens (three 2048/1408-contraction
matmul phases in bf16 with fp32 PSUM accumulation, feature-major layout so
no on-device transposes are needed), and the host scatter-adds the
per-expert outputs back into the full [T, H] result.

Weights are pre-tiled on the host into [n_tiles, 128, contract*128] blocks
so every DMA moves 4 KiB contiguous per partition (vs 256 B chunks when
slicing the natural [H, I] layout — measured 2x DMA throughput difference).

Per-core capacity C = max tokens routed to any expert, rounded up to 16;
pad slots carry combine-weight 0, so their contribution is exactly zero.
"""

import numpy as np
import ml_dtypes


def _ensure_ntff_hook():
    """This image's antenv package lacks axon_hooks, but concourse's
    run_bass_kernel_spmd unconditionally imports it when BASS_TRACE is set.
    Provide the module (and the ctypes NTFF hook from trn_agent_boot, when
    available) so tracing works instead of crashing. Idempotent; never
    overwrites an existing module."""
    import sys
    import types
    try:
        import antenv  # noqa: F401
    except ImportError:
        return
    if "antenv.axon_hooks" in sys.modules:
        return
    try:
        import antenv.axon_hooks  # noqa: F401
        return
    except ImportError:
        pass
    mod = types.ModuleType("antenv.axon_hooks")
    holder = {"h": None}
    mod.set_axon_ntff_profile_hook = lambda h: holder.__setitem__("h", h)
    mod.get_axon_ntff_profile_hook = lambda: holder.get("h")
    sys.modules["antenv.axon_hooks"] = mod
    import antenv as _a
    _a.axon_hooks = mod
    try:
        from trn_agent_boot.trn_boot import _ntff_profile_via_ctypes
        hook = _ntff_profile_via_ctypes("/opt/axon/libaxon_pjrt.so")
        if hook is not None:
            mod.set_axon_ntff_profile_hook(hook)
    except Exception:
        pass


_ensure_ntff_hook()

H = 2048
I_DIM = 1408
E = 8
P = 128
HT = H // P      # 16
IT = I_DIM // P  # 11

_compiled = {}
last_results = None


def _chunks(C):
    """Token-column chunks of <=512 (one PSUM bank / max moving free dim)."""
    out = []
    s = 0
    while s < C:
        w = min(512, C - s)
        out.append((s, w))
        s += w
    return out


def _build(C):
    import concourse.bacc as bacc
    import concourse.mybir as mybir
    import concourse.tile as tile

    dt = mybir.dt
    nc = bacc.Bacc("TRN2", target_bir_lowering=False)
    # Pre-tiled weight layouts: wg/wu [IT, 128, HT*128], wd [HT, 128, IT*128].
    # Block [t, p, k*128+c] = W[k*128+p, t*128+c] of the natural layout, i.e.
    # partition p of block t holds that block's full contraction row,
    # contiguous in DRAM.
    xg = nc.dram_tensor("xg", [P, HT * C], dt.bfloat16, kind="ExternalInput")
    wt = nc.dram_tensor("wt", [P, C], dt.float32, kind="ExternalInput")
    wg = nc.dram_tensor("wg", [IT, P, HT * P], dt.bfloat16, kind="ExternalInput")
    wu = nc.dram_tensor("wu", [IT, P, HT * P], dt.bfloat16, kind="ExternalInput")
    wd = nc.dram_tensor("wd", [HT, P, IT * P], dt.bfloat16, kind="ExternalInput")
    yt = nc.dram_tensor("yt", [H, C], dt.float32, kind="ExternalOutput")

    ch = _chunks(C)

    with tile.TileContext(nc) as tc:
        with (
            tc.tile_pool(name="xpool", bufs=1) as xpool,
            tc.tile_pool(name="apool", bufs=1) as apool,
            tc.tile_pool(name="wpool", bufs=3) as wpool,
            tc.tile_pool(name="wdpool", bufs=4) as wdpool,
            tc.tile_pool(name="spool", bufs=2) as spool,
            tc.tile_pool(name="ypool", bufs=3) as ypool,
        ):
            def load_w(pool, src, t, tag, eng=None):
                # phase-1 weight DMAs trigger on GpSimd, keeping the Sync
                # sequencer free for token loads (each trigger serializes
                # ~0.6us on its issuing engine's sequencer)
                w_t = pool.tile([P, HT if src is not wd else IT, P],
                                dt.bfloat16, name=tag, tag=tag)
                (eng or nc.gpsimd).dma_start(out=w_t[:], in_=src[t, :, :])
                return w_t

            # Head ordering on the Sync HWDGE, in first-use order. xg is
            # host-packed [128, HT*C] (partition rows contiguous in DRAM) and
            # loaded as four SEPARATE quarter tiles — one DMA per tile keeps
            # the h=0..3 matmuls' dependency exactly "quarter 0 landed"
            # instead of a conservative wait on the whole token block.
            wb = xpool.tile([P, C], dt.float32, name="wb", tag="wb")
            HQ = HT // 4
            xq_t = []

            def load_xq(q):
                t = xpool.tile([P, HQ * C], dt.bfloat16, name=f"xq{q}", tag=f"xq{q}")
                nc.sync.dma_start(out=t[:], in_=xg[:, q * HQ * C:(q + 1) * HQ * C])
                xq_t.append(t)

            wgt0 = load_w(wpool, wg, 0, "wg", eng=nc.sync)
            load_xq(0)
            wut0 = load_w(wpool, wu, 0, "wu", eng=nc.sync)
            load_xq(1)
            nc.sync.dma_start(out=wb[:], in_=wt[:, :])
            load_xq(2)
            load_xq(3)
            xg_t = [xq_t[h // HQ][:, (h % HQ) * C:(h % HQ + 1) * C]
                    for h in range(HT)]

            # PE warm-up while token DMAs stream: ~6us of tiny matmuls on a
            # zeroed scratch tile releases the HAM clock gate (1.2 -> 2.4 GHz
            # takes ~3.4us of sustained PE activity) before real work lands.
            warm = spool.tile([P, 64], dt.bfloat16, name="warm", tag="warm")
            nc.vector.memset(warm[:], 0.0)

            # Phase 1: A[i, t] = silu(G) * U, feature-major, per 128-row i-tile.
            a_t = []
            with tc.tile_pool(name="pp1", bufs=2, space="PSUM") as pp1:
                for it in range(IT):
                    if it == 0:
                        wgt, wut = wgt0, wut0
                    else:
                        wgt = load_w(wpool, wg, it, "wg")
                        wut = load_w(wpool, wu, it, "wu")
                    pgs = [pp1.tile([P, w], dt.float32, name=f"pg{ci}", tag=f"pg{ci}",
                                    bufs=2 if ci == 0 else 1)
                           for ci, (s, w) in enumerate(ch)]
                    pus = [pp1.tile([P, w], dt.float32, name=f"pu{ci}", tag=f"pu{ci}",
                                    bufs=2 if ci == 0 else 1)
                           for ci, (s, w) in enumerate(ch)]
                    if it == 0:
                        for _ in range(128):
                            nc.tensor.matmul(pgs[0][:64, :64], warm[:, :], warm[:, :64],
                                             start=True, stop=True)
                    for h in range(HT):
                        st, sp = h == 0, h == HT - 1
                        for ci, (s, w) in enumerate(ch):
                            nc.tensor.matmul(pgs[ci][:], wgt[:, h, :],
                                             xg_t[h][:, s:s + w], start=st, stop=sp)
                        for ci, (s, w) in enumerate(ch):
                            nc.tensor.matmul(pus[ci][:], wut[:, h, :],
                                             xg_t[h][:, s:s + w], start=st, stop=sp)
                    sg = spool.tile([P, C], dt.float32, name="sg", tag="sg")
                    ai = apool.tile([P, C], dt.bfloat16, name=f"a{it}", tag=f"a{it}")
                    for ci, (s, w) in enumerate(ch):
                        nc.scalar.activation(sg[:, s:s + w], pgs[ci][:],
                                             mybir.ActivationFunctionType.Silu)
                        nc.vector.tensor_mul(ai[:, s:s + w], sg[:, s:s + w], pus[ci][:])
                    a_t.append(ai)

            # Phase 2: Y^T[h, t] = sum_i Wd[i, h] * A[i, t].
            with tc.tile_pool(name="pp2", bufs=2, space="PSUM") as pp2:
                for ht in range(HT):
                    wdt = load_w(wdpool, wd, ht, "wd", eng=nc.sync)
                    pys = [pp2.tile([P, w], dt.float32, name=f"py{ci}", tag=f"py{ci}")
                           for ci, (s, w) in enumerate(ch)]
                    for i2 in range(IT):
                        st, sp = i2 == 0, i2 == IT - 1
                        for ci, (s, w) in enumerate(ch):
                            nc.tensor.matmul(pys[ci][:], wdt[:, i2, :],
                                             a_t[i2][:, s:s + w], start=st, stop=sp)
                    yo = ypool.tile([P, C], dt.float32, name="yo", tag="yo")
                    for ci, (s, w) in enumerate(ch):
                        nc.vector.tensor_mul(yo[:, s:s + w], wb[:, s:s + w], pys[ci][:])
                    nc.scalar.dma_start(out=yt[ht * P:(ht + 1) * P, :], in_=yo[:])
    nc.compile()
    return nc


def _tile_weight(w, nt_out):
    """[K, N] -> [N/128, 128, K] blocks: out[t, p, k*128+c] = w[k*128+p, t*128+c]."""
    K, N = w.shape
    kt = K // P
    return np.ascontiguousarray(
        w.reshape(kt, P, nt_out, P).transpose(2, 1, 0, 3).reshape(nt_out, P, kt * P)
    )


def kernel(hidden_states, gate_w, w_gate, w_up, w_down, top_k):
    global last_results
    hs = np.ascontiguousarray(np.asarray(hidden_states, dtype=np.float32))
    gw = np.asarray(gate_w, dtype=np.float32)
    wg_all = np.asarray(w_gate, dtype=np.float32)
    wu_all = np.asarray(w_up, dtype=np.float32)
    wd_all = np.asarray(w_down, dtype=np.float32)
    K = int(np.asarray(top_k))
    T = hs.shape[0]
    if K <= 0:
        return np.zeros((T, H), np.float32)

    # ---- router (mirrors the reference numerics in fp32) ----
    logits = hs @ gw.T
    m = logits.max(-1, keepdims=True)
    ex = np.exp(logits - m)
    probs = ex / ex.sum(-1, keepdims=True)
    order = np.argsort(-probs, axis=-1, kind="stable")
    topi = order[:, :K]
    topv = np.take_along_axis(probs, topi, axis=-1)
    topv = topv / topv.sum(-1, keepdims=True)

    # ---- dispatch: gather each expert's tokens ----
    idxs, wvs = [], []
    for e in range(E):
        mask = topi == e
        rows = np.nonzero(mask.any(-1))[0]
        idxs.append(rows)
        wvs.append(topv[mask].astype(np.float32))
    counts = [len(r) for r in idxs]
    C = max(64, ((max(counts) + 1) // 2) * 2)

    nc = _compiled.get(C)
    if nc is None:
        nc = _compiled[C] = _build(C)

    bf16 = ml_dtypes.bfloat16
    in_maps = []
    for e in range(E):
        idx, wv = idxs[e], wvs[e]
        n = len(idx)
        xsel = hs[idx]  # [n, H]
        xg_np = np.zeros((HT, P, C), dtype=bf16)
        xg_np[:, :, :n] = xsel.T.astype(bf16).reshape(HT, P, n)
        xg_np = np.ascontiguousarray(xg_np.transpose(1, 0, 2).reshape(P, HT * C))
        wt_np = np.zeros((P, C), dtype=np.float32)
        wt_np[:, :n] = wv[None, :]
        in_maps.append({
            "xg": xg_np,
            "wt": wt_np,
            "wg": _tile_weight(wg_all[e].astype(bf16), IT),
            "wu": _tile_weight(wu_all[e].astype(bf16), IT),
            "wd": _tile_weight(wd_all[e].astype(bf16), HT),
        })

    from concourse.bass_utils import run_bass_kernel_spmd
    res = run_bass_kernel_spmd(nc, in_maps, core_ids=list(range(E)))
    last_results = res

    # ---- combine: scatter-add per-expert outputs ----
    out = np.zeros((T, H), np.float32)
    for e in range(E):
        idx = idxs[e]
        n = len(idx)
        yt_e = res.results[e]["yt"]  # [H, C] fp32
        out[idx] += yt_e[:, :n].T
    return out


# revision 16
# speedup vs baseline: 1.0230x; 1.0230x over previous
"""DeepSeek-V2 normal MoE layer on 8 Trainium2 NeuronCores.

Expert-parallel sharding: core e holds expert e's weights (cast to bf16).
The router (tiny [T,E] matmul + softmax + top-k) runs on the host in fp32 —
this is the dispatch step of the sharding layer: it decides which token rows
are copied to which core. Each core receives its routed tokens twice
(plain, and pre-scaled by the renormalized top-k combine weight, which folds
the output weighting into the linear up-projection). On device, each core
computes gated-SiLU expert MLP for its tokens (three 2048/1408-contraction
matmul phases in bf16 with fp32 PSUM accumulation, feature-major layout so
no on-device transposes are needed), and the host scatter-adds the
per-expert outputs back into the full [T, H] result.

Weights are pre-tiled on the host into [n_tiles, 128, contract*128] blocks
so every DMA moves 4 KiB contiguous per partition (vs 256 B chunks when
slicing the natural [H, I] layout — measured 2x DMA throughput difference).

Per-core capacity C = max tokens routed to any expert, rounded up to 16;
pad slots carry combine-weight 0, so their contribution is exactly zero.
"""

import numpy as np
import ml_dtypes


def _ensure_ntff_hook():
    """This image's antenv package lacks axon_hooks, but concourse's
    run_bass_kernel_spmd unconditionally imports it when BASS_TRACE is set.
    Provide the module (and the ctypes NTFF hook from trn_agent_boot, when
    available) so tracing works instead of crashing. Idempotent; never
    overwrites an existing module."""
    import sys
    import types
    try:
        import antenv  # noqa: F401
    except ImportError:
        return
    if "antenv.axon_hooks" in sys.modules:
        return
    try:
        import antenv.axon_hooks  # noqa: F401
        return
    except ImportError:
        pass
    mod = types.ModuleType("antenv.axon_hooks")
    holder = {"h": None}
    mod.set_axon_ntff_profile_hook = lambda h: holder.__setitem__("h", h)
    mod.get_axon_ntff_profile_hook = lambda: holder.get("h")
    sys.modules["antenv.axon_hooks"] = mod
    import antenv as _a
    _a.axon_hooks = mod
    try:
        from trn_agent_boot.trn_boot import _ntff_profile_via_ctypes
        hook = _ntff_profile_via_ctypes("/opt/axon/libaxon_pjrt.so")
        if hook is not None:
            mod.set_axon_ntff_profile_hook(hook)
    except Exception:
        pass


_ensure_ntff_hook()

H = 2048
I_DIM = 1408
E = 8
P = 128
HT = H // P      # 16
IT = I_DIM // P  # 11

_compiled = {}
last_results = None


def _chunks(C):
    """Token-column chunks of <=512 (one PSUM bank / max moving free dim)."""
    out = []
    s = 0
    while s < C:
        w = min(512, C - s)
        out.append((s, w))
        s += w
    return out


def _build(C):
    import concourse.bacc as bacc
    import concourse.mybir as mybir
    import concourse.tile as tile

    dt = mybir.dt
    nc = bacc.Bacc("TRN2", target_bir_lowering=False)
    # Pre-tiled weight layouts: wg/wu [IT, 128, HT*128], wd [HT, 128, IT*128].
    # Block [t, p, k*128+c] = W[k*128+p, t*128+c] of the natural layout, i.e.
    # partition p of block t holds that block's full contraction row,
    # contiguous in DRAM.
    xg = nc.dram_tensor("xg", [P, HT * C], dt.bfloat16, kind="ExternalInput")
    wt = nc.dram_tensor("wt", [P, C], dt.float32, kind="ExternalInput")
    wg = nc.dram_tensor("wg", [IT, P, HT * P], dt.bfloat16, kind="ExternalInput")
    wu = nc.dram_tensor("wu", [IT, P, HT * P], dt.bfloat16, kind="ExternalInput")
    wd = nc.dram_tensor("wd", [HT, P, IT * P], dt.bfloat16, kind="ExternalInput")
    yt = nc.dram_tensor("yt", [H, C], dt.float32, kind="ExternalOutput")

    ch = _chunks(C)

    with tile.TileContext(nc) as tc:
        with (
            tc.tile_pool(name="xpool", bufs=1) as xpool,
            tc.tile_pool(name="apool", bufs=1) as apool,
            tc.tile_pool(name="wpool", bufs=3) as wpool,
            tc.tile_pool(name="wdpool", bufs=4) as wdpool,
            tc.tile_pool(name="spool", bufs=2) as spool,
            tc.tile_pool(name="ypool", bufs=3) as ypool,
        ):
            def load_w(pool, src, t, tag, eng=None):
                # phase-1 weight DMAs trigger on GpSimd, keeping the Sync
                # sequencer free for token loads (each trigger serializes
                # ~0.6us on its issuing engine's sequencer)
                w_t = pool.tile([P, HT if src is not wd else IT, P],
                                dt.bfloat16, name=tag, tag=tag)
                (eng or nc.gpsimd).dma_start(out=w_t[:], in_=src[t, :, :])
                return w_t

            # Head ordering on the Sync HWDGE, in first-use order. xg is
            # host-packed [128, HT*C] (partition rows contiguous in DRAM) and
            # loaded as four SEPARATE quarter tiles — one DMA per tile keeps
            # the h=0..3 matmuls' dependency exactly "quarter 0 landed"
            # instead of a conservative wait on the whole token block.
            wb = xpool.tile([P, C], dt.float32, name="wb", tag="wb")
            HQ = HT // 4
            xq_t = []

            def load_xq(q):
                t = xpool.tile([P, HQ * C], dt.bfloat16, name=f"xq{q}", tag=f"xq{q}")
                nc.sync.dma_start(out=t[:], in_=xg[:, q * HQ * C:(q + 1) * HQ * C])
                xq_t.append(t)

            wgt0 = load_w(wpool, wg, 0, "wg", eng=nc.sync)
            load_xq(0)
            wut0 = load_w(wpool, wu, 0, "wu", eng=nc.sync)
            load_xq(1)
            nc.sync.dma_start(out=wb[:], in_=wt[:, :])
            load_xq(2)
            load_xq(3)
            xg_t = [xq_t[h // HQ][:, (h % HQ) * C:(h % HQ + 1) * C]
                    for h in range(HT)]

            # PE warm-up while token DMAs stream: ~6us of tiny matmuls on a
            # zeroed scratch tile releases the HAM clock gate (1.2 -> 2.4 GHz
            # takes ~3.4us of sustained PE activity) before real work lands.
            warm = spool.tile([P, 64], dt.bfloat16, name="warm", tag="warm")
            nc.vector.memset(warm[:], 0.0)

            # Phase 1: A[i, t] = silu(G) * U, feature-major, per 128-row i-tile.
            a_t = []
            with tc.tile_pool(name="pp1", bufs=2, space="PSUM") as pp1:
                for it in range(IT):
                    if it == 0:
                        wgt, wut = wgt0, wut0
                    else:
                        wgt = load_w(wpool, wg, it, "wg")
                        wut = load_w(wpool, wu, it, "wu")
                    pgs = [pp1.tile([P, w], dt.float32, name=f"pg{ci}", tag=f"pg{ci}",
                                    bufs=2 if ci == 0 else 1)
                           for ci, (s, w) in enumerate(ch)]
                    pus = [pp1.tile([P, w], dt.float32, name=f"pu{ci}", tag=f"pu{ci}",
                                    bufs=2 if ci == 0 else 1)
                           for ci, (s, w) in enumerate(ch)]
                    if it == 0:
                        for _ in range(48):
                            nc.tensor.matmul(pgs[0][:64, :64], warm[:, :], warm[:, :64],
                                             start=True, stop=True)
                    for h in range(HT):
                        st, sp = h == 0, h == HT - 1
                        for ci, (s, w) in enumerate(ch):
                            nc.tensor.matmul(pgs[ci][:], wgt[:, h, :],
                                             xg_t[h][:, s:s + w], start=st, stop=sp)
                        for ci, (s, w) in enumerate(ch):
                            nc.tensor.matmul(pus[ci][:], wut[:, h, :],
                                             xg_t[h][:, s:s + w], start=st, stop=sp)
                    sg = spool.tile([P, C], dt.float32, name="sg", tag="sg")
                    ai = apool.tile([P, C], dt.bfloat16, name=f"a{it}", tag=f"a{it}")
                    for ci, (s, w) in enumerate(ch):
                        nc.scalar.activation(sg[:, s:s + w], pgs[ci][:],
                                             mybir.ActivationFunctionType.Silu)
                        nc.vector.tensor_mul(ai[:, s:s + w], sg[:, s:s + w], pus[ci][:])
                    a_t.append(ai)

            # Phase 2: Y^T[h, t] = sum_i Wd[i, h] * A[i, t].
            with tc.tile_pool(name="pp2", bufs=2, space="PSUM") as pp2:
                for ht in range(HT):
                    wdt = load_w(wdpool, wd, ht, "wd", eng=nc.sync)
                    pys = [pp2.tile([P, w], dt.float32, name=f"py{ci}", tag=f"py{ci}")
                           for ci, (s, w) in enumerate(ch)]
                    for i2 in range(IT):
                        st, sp = i2 == 0, i2 == IT - 1
                        for ci, (s, w) in enumerate(ch):
                            nc.tensor.matmul(pys[ci][:], wdt[:, i2, :],
                                             a_t[i2][:, s:s + w], start=st, stop=sp)
                    yo = ypool.tile([P, C], dt.float32, name="yo", tag="yo")
                    for ci, (s, w) in enumerate(ch):
                        nc.vector.tensor_mul(yo[:, s:s + w], wb[:, s:s + w], pys[ci][:])
                    nc.scalar.dma_start(out=yt[ht * P:(ht + 1) * P, :], in_=yo[:])
    nc.compile()
    return nc


def _tile_weight(w, nt_out):
    """[K, N] -> [N/128, 128, K] blocks: out[t, p, k*128+c] = w[k*128+p, t*128+c]."""
    K, N = w.shape
    kt = K // P
    return np.ascontiguousarray(
        w.reshape(kt, P, nt_out, P).transpose(2, 1, 0, 3).reshape(nt_out, P, kt * P)
    )


def kernel(hidden_states, gate_w, w_gate, w_up, w_down, top_k):
    global last_results
    hs = np.ascontiguousarray(np.asarray(hidden_states, dtype=np.float32))
    gw = np.asarray(gate_w, dtype=np.float32)
    wg_all = np.asarray(w_gate, dtype=np.float32)
    wu_all = np.asarray(w_up, dtype=np.float32)
    wd_all = np.asarray(w_down, dtype=np.float32)
    K = int(np.asarray(top_k))
    T = hs.shape[0]
    if K <= 0:
        return np.zeros((T, H), np.float32)

    # ---- router (mirrors the reference numerics in fp32) ----
    logits = hs @ gw.T
    m = logits.max(-1, keepdims=True)
    ex = np.exp(logits - m)
    probs = ex / ex.sum(-1, keepdims=True)
    order = np.argsort(-probs, axis=-1, kind="stable")
    topi = order[:, :K]
    topv = np.take_along_axis(probs, topi, axis=-1)
    topv = topv / topv.sum(-1, keepdims=True)

    # ---- dispatch: gather each expert's tokens ----
    idxs, wvs = [], []
    for e in range(E):
        mask = topi == e
        rows = np.nonzero(mask.any(-1))[0]
        idxs.append(rows)
        wvs.append(topv[mask].astype(np.float32))
    counts = [len(r) for r in idxs]
    C = max(64, ((max(counts) + 1) // 2) * 2)

    nc = _compiled.get(C)
    if nc is None:
        nc = _compiled[C] = _build(C)

    bf16 = ml_dtypes.bfloat16
    in_maps = []
    for e in range(E):
        idx, wv = idxs[e], wvs[e]
        n = len(idx)
        xsel = hs[idx]  # [n, H]
        xg_np = np.zeros((HT, P, C), dtype=bf16)
        xg_np[:, :, :n] = xsel.T.astype(bf16).reshape(HT, P, n)
        xg_np = np.ascontiguousarray(xg_np.transpose(1, 0, 2).reshape(P, HT * C))
        wt_np = np.zeros((P, C), dtype=np.float32)
        wt_np[:, :n] = wv[None, :]
        in_maps.append({
            "xg": xg_np,
            "wt": wt_np,
            "wg": _tile_weight(wg_all[e].astype(bf16), IT),
            "wu": _tile_weight(wu_all[e].astype(bf16), IT),
            "wd": _tile_weight(wd_all[e].astype(bf16), HT),
        })

    from concourse.bass_utils import run_bass_kernel_spmd
    res = run_bass_kernel_spmd(nc, in_maps, core_ids=list(range(E)))
    last_results = res

    # ---- combine: scatter-add per-expert outputs ----
    out = np.zeros((T, H), np.float32)
    for e in range(E):
        idx = idxs[e]
        n = len(idx)
        yt_e = res.results[e]["yt"]  # [H, C] fp32
        out[idx] += yt_e[:, :n].T
    return out
